# revision 20
# baseline (speedup 1.0000x reference)
"""Causal self-attention (b=2, n=2048, d_model=1024, 16 heads) on 8 TRN2 cores.

Sharding: core c handles batch c//4, heads 4*(c%4) .. 4*(c%4)+3 (data parallel
over batch x tensor parallel over heads). Each core computes its 4 heads'
attention and a partial output projection; the host sums the 4 partials per
batch (partials stored bf16, summed in f32 on host).

Device pipeline per core (all matmuls bf16 with fp32 PSUM accumulation):
  A. x^T arrives host-transposed; qT/kT [2x128, 2048] = W^T x^T feature-major;
     V [2048, 4x64] token-major padded with 64 ones-columns per head so the
     AV matmul also emits replicated softmax row-sums.
  B. Attention runs qb-outer (512-query blocks), fc-inner (head pairs).
     S^T tiles [128k, 512q] via row-paired K=64 matmuls; causally-trimmed:
     diagonal key chunks only compute q >= chunk start (N = 512-128j), the
     per-element diagonal triangle handled by a single [128,128] tril mask
     multiplied into the 128x128 diagonal blocks of the prob tiles.
     exp on ScalarE (scale=1/8, no max-subtraction: |scores/8| << 88).
  C. PE filler (remaining qk/V groups, then per-token-block projection units)
     keeps the PE stream dense while ACT paces the S->exp->AV chain.
  D. Projection per 128-token block t: Y_t = oT_t.T @ Wp (both 512-col
     halves), cast bf16, one DMA per block on the idle Sync queue.
"""
import contextlib
import ctypes
import os
import sys
import types

import numpy as np
import ml_dtypes

import concourse.bacc as bacc
import concourse.tile as tile
from concourse import mybir
from concourse.bass_utils import run_bass_kernel_spmd

F32 = mybir.dt.float32
BF16 = mybir.dt.bfloat16
AF = mybir.ActivationFunctionType
ALU = mybir.AluOpType

N = 2048          # sequence length
D = 1024          # d_model
NH = 16           # total heads
HD = 64           # head dim
HPC = 4           # heads per core
NCORES = 8
TC = N // 128     # token chunks (16)
KC = D // 128     # d_model chunks (8)
QB = N // 512     # 512-wide q blocks (4)

_BF16 = ml_dtypes.bfloat16

_nc_cache = None
LAST_EXEC_NS = None


def _install_ntff_hook():
    """bass_utils wants antenv.axon_hooks for trace=True under axon; the agent
    image lacks it. Synthesize it (same ctypes recipe trn_agent_boot uses)."""
    if "antenv.axon_hooks" in sys.modules:
        return
    so_path = "/opt/axon/libaxon_pjrt.so"
    try:
        lib = ctypes.CDLL(so_path)
        lib.axon_start_nrt_profile.argtypes = [
            ctypes.POINTER(ctypes.c_int64), ctypes.c_size_t]
        lib.axon_start_nrt_profile.restype = ctypes.c_int64
        lib.axon_stop_nrt_profile.argtypes = [ctypes.c_char_p]
        lib.axon_stop_nrt_profile.restype = ctypes.c_int64
    except OSError:
        return

    @contextlib.contextmanager
    def _hook(output_dir, device_ids):
        import jax
        jax.devices()
        if device_ids:
            ids = (ctypes.c_int64 * len(device_ids))(*device_ids)
            rc = lib.axon_start_nrt_profile(ids, len(device_ids))
        else:
            rc = lib.axon_start_nrt_profile(None, 0)
        if rc != 0:
            raise RuntimeError(f"axon_start_nrt_profile rc={rc}")
        try:
            yield
        finally:
            n = lib.axon_stop_nrt_profile(str(output_dir).encode())
            print(f"ntff profile: {n} file(s) -> {output_dir}", file=sys.stderr)

    mod = types.ModuleType("antenv.axon_hooks")
    mod.get_axon_ntff_profile_hook = lambda: _hook
    mod.set_axon_ntff_profile_hook = lambda h: None
    sys.modules["antenv.axon_hooks"] = mod
    try:
        import antenv
        antenv.axon_hooks = mod
    except ImportError:
        pass


def _build_nc():
    nc = bacc.Bacc("TRN2", target_bir_lowering=False, debug=False)
    # weights arrive host-pre-chunked as [128, c*f] so the DMA reads 4KB
    # contiguous per partition row (512B rows ran at ~84GB/s, 4x slower)
    x_d = nc.dram_tensor("x", [D, N], BF16, kind="ExternalInput")  # x^T, host-transposed
    wq_d = nc.dram_tensor("wq", [128, KC * HPC * HD], BF16, kind="ExternalInput")
    wk_d = nc.dram_tensor("wk", [128, KC * HPC * HD], BF16, kind="ExternalInput")
    wv_d = nc.dram_tensor("wv", [128, KC * HPC * HD], BF16, kind="ExternalInput")
    wp_d = nc.dram_tensor("wp", [128, 2 * D], BF16, kind="ExternalInput")
    y_d = nc.dram_tensor("y", [N, D], BF16, kind="ExternalOutput")

    with tile.TileContext(nc) as tc:
        with (
            tc.tile_pool(name="const", bufs=1) as constp,
            tc.tile_pool(name="big", bufs=1) as big,
            tc.tile_pool(name="work", bufs=3) as work,
        ):
            # persistent SBUF tensors
            xT = big.tile([128, KC, N], BF16, tag="xT")          # xT[p,d,t] = x[t, d*128+p]
            qT = big.tile([128, 2, N], BF16, tag="qT")           # [head-pair chunk][feat, tok]
            kT = big.tile([128, 2, N], BF16, tag="kT")
            vv = big.tile([128, TC, HPC, 128], BF16, tag="vv")   # V' per head: 64 V cols + 64 ones
            oT = big.tile([128, 2, N], BF16, tag="oT")           # normalized O^T
            wq_s = big.tile([128, KC, HPC * HD], BF16, tag="wq")
            wk_s = big.tile([128, KC, HPC * HD], BF16, tag="wk")
            wv_s = big.tile([128, KC, HPC * HD], BF16, tag="wv")
            wp_s = big.tile([128, 2, D], BF16, tag="wp")

            # Input DMAs are HBM-bandwidth-bound (~16us total); order the wire
            # stream so the chunk-major prologue can chase x chunks: qkv
            # weights first, then x chunks round-robin on the two HWDGE
            # queues, wp (needed late) last on gpsimd's SW queue.
            x_r = x_d.ap().rearrange("(c p) t -> p c t", p=128)
            nc.sync.dma_start(wq_s[:], wq_d.ap().rearrange("p (c f) -> p c f", c=KC))
            nc.scalar.dma_start(wk_s[:], wk_d.ap().rearrange("p (c f) -> p c f", c=KC))
            nc.gpsimd.dma_start(wv_s[:], wv_d.ap().rearrange("p (c f) -> p c f", c=KC))
            for c in range(KC):
                eng = nc.sync if c % 2 == 0 else nc.scalar
                eng.dma_start(xT[:, c, :], x_r[:, c, :])
            nc.gpsimd.memset(vv[:, :, :, HD:], 1.0)  # ones columns -> replicated row-sums

            # single [128,128] tril mask for the per-element diagonal blocks:
            # dmask[k, q] = 1 iff q >= k
            dmask = constp.tile([128, 128], BF16, tag="dmask")
            nc.gpsimd.memset(dmask[:], 1.0)
            nc.gpsimd.affine_select(
                out=dmask[:], in_=dmask[:], compare_op=ALU.is_ge, fill=0.0,
                base=0, pattern=[[1, 128]], channel_multiplier=-1,
            )
            nc.gpsimd.dma_start(wp_s[:], wp_d.ap().rearrange("p (c f) -> p c f", c=2))

            # ---- chunk-major prologue: everything attention(qb0) needs ----
            # 8 accumulation groups live in 8 PSUM banks simultaneously; the
            # per-chunk MMs chase the x DMA arrivals instead of head-blocking
            # the in-order PE queue on the last chunk.
            with tc.tile_pool(name="psP", bufs=1, space="PSUM") as psP:
                pq = [psP.tile([128, 512], F32, tag=f"pq{i}", name=f"pq{i}")
                      for i in range(4)]
                pv = [psP.tile([128, HPC * HD], F32, tag=f"pv{i}", name=f"pv{i}")
                      for i in range(3)]
                qk_list = [(wq_s, 0), (wk_s, 0), (wq_s, 1), (wk_s, 1)]
                for kc in range(KC):
                    for i, (ws, fc) in enumerate(qk_list):
                        nc.tensor.matmul(
                            pq[i][:], ws[:, kc, fc * 128:(fc + 1) * 128],
                            xT[:, kc, 0:512],
                            start=(kc == 0), stop=(kc == KC - 1),
                        )
                    for t in range(3):
                        nc.tensor.matmul(
                            pv[t][:], xT[:, kc, t * 128:(t + 1) * 128],
                            wv_s[:, kc, :],
                            start=(kc == 0), stop=(kc == KC - 1),
                        )
                for i, dst in enumerate((qT, kT)):
                    nc.vector.tensor_copy(dst[:, 0, 0:512], pq[i][:])
                    nc.vector.tensor_copy(dst[:, 1, 0:512], pq[i + 2][:])
                for t in range(3):
                    nc.vector.tensor_copy(
                        vv[:, t, :, 0:HD],
                        pv[t][:].rearrange("p (h d) -> p h d", h=HPC),
                    )

            with (
                tc.tile_pool(name="psS", bufs=2, space="PSUM") as psS,
                tc.tile_pool(name="psO", bufs=2, space="PSUM") as psO,
                tc.tile_pool(name="psF", bufs=2, space="PSUM") as psF,
            ):
                def emit_qk_group(fc, tb, wsrc, dst):
                    ps = psF.tile([128, 512], F32, tag="fy")
                    for kc in range(KC):
                        nc.tensor.matmul(
                            ps[:], wsrc[:, kc, fc * 128:(fc + 1) * 128],
                            xT[:, kc, tb * 512:(tb + 1) * 512],
                            start=(kc == 0), stop=(kc == KC - 1),
                        )
                    nc.vector.tensor_copy(dst[:, fc, tb * 512:(tb + 1) * 512], ps[:])

                def emit_v_group(t):
                    ps = psF.tile([128, HPC * HD], F32, tag="fy")
                    for kc in range(KC):
                        nc.tensor.matmul(
                            ps[:], xT[:, kc, t * 128:(t + 1) * 128], wv_s[:, kc, :],
                            start=(kc == 0), stop=(kc == KC - 1),
                        )
                    nc.vector.tensor_copy(
                        vv[:, t, :, 0:HD],
                        ps[:].rearrange("p (h d) -> p h d", h=HPC),
                    )

                drain_mode = [False]
                ys_live = {}

                def emit_proj_half(t, nh):
                    # one 512-col half of Y for token block t; DMA after nh=1
                    if nh == 0:
                        ys_live[t] = work.tile([128, D], BF16, tag="ys", name="ys")
                    ys = ys_live[t]
                    ps = psF.tile([128, 512], F32, tag="fy")
                    for fc in range(2):
                        nc.tensor.matmul(
                            ps[:], oT[:, fc, t * 128:(t + 1) * 128],
                            wp_s[:, fc, nh * 512:(nh + 1) * 512],
                            start=(fc == 0), stop=(fc == 1),
                        )
                    dst = ys[:, nh * 512:(nh + 1) * 512]
                    if drain_mode[0]:
                        # post-attention drain: ScalarE is idle
                        nc.scalar.activation(dst, ps[:], AF.Copy)
                    else:
                        nc.vector.tensor_copy(dst, ps[:])
                    if nh == 1:
                        del ys_live[t]
                        nc.sync.dma_start(
                            y_d.ap()[t * 128:(t + 1) * 128, :], ys[:])

                # filler ordered so every entry is emitted before its first
                # consumer under the per-(qb,fc) pop counts in NPOP below
                filler = [
                    (emit_v_group, (3,)),
                    (emit_qk_group, (0, 1, wq_s, qT)),
                    (emit_qk_group, (0, 1, wk_s, kT)),
                    (emit_qk_group, (1, 1, wq_s, qT)),
                    (emit_qk_group, (1, 1, wk_s, kT)),
                    (emit_v_group, (4,)), (emit_v_group, (5,)),
                    (emit_v_group, (6,)), (emit_v_group, (7,)),
                    (emit_qk_group, (0, 2, wq_s, qT)),
                    (emit_qk_group, (0, 2, wk_s, kT)),
                    (emit_qk_group, (1, 2, wq_s, qT)),
                    (emit_qk_group, (1, 2, wk_s, kT)),
                    (emit_v_group, (8,)), (emit_v_group, (9,)),
                    (emit_v_group, (10,)), (emit_v_group, (11,)),
                    (emit_qk_group, (0, 3, wq_s, qT)),
                    (emit_qk_group, (0, 3, wk_s, kT)),
                    (emit_v_group, (12,)), (emit_v_group, (13,)),
                    (emit_v_group, (14,)), (emit_v_group, (15,)),
                    (emit_qk_group, (1, 3, wq_s, qT)),
                    (emit_qk_group, (1, 3, wk_s, kT)),
                ]

                def pops(n):
                    for _ in range(min(n, len(filler))):
                        fn, args = filler.pop(0)
                        fn(*args)

                # pops per AV-iteration, keyed (qb, fc).  Tuned so the filler
                # stream covers the ACT-paced deficit early, does not run dry
                # during qb3, and leaves ~3 proj halves for the drain: those
                # are emitted after the final normalize but do not depend on
                # it, so PE stays busy (and the HAM clock warm) through it.
                def npop_for(qb, fc, it):
                    if (qb, fc) == (3, 1):
                        return 2 if 3 <= it <= 7 else 0
                    if qb == 0:
                        return 2
                    return 1

                for qb in range(QB):
                    nkc = 4 * (qb + 1)
                    npr = nkc // 2
                    qs = slice(qb * 512, (qb + 1) * 512)
                    for fc in range(2):
                        oA = psO.tile([128, 512], F32, tag="oy")
                        oB = psO.tile([128, 512], F32, tag="oy")
                        aAs, aBs = {}, {}
                        for it in range(npr + 1):
                            if it < npr:
                                sA = psS.tile([128, 1024], F32, tag="s")
                                sB = psS.tile([128, 1024], F32, tag="s")
                                for half in range(2):
                                    kc = 2 * it + half
                                    j = kc - 4 * qb
                                    off = 128 * j if j > 0 else 0
                                    hs = slice(half * 512 + off, (half + 1) * 512)
                                    qsl = slice(qb * 512 + off, (qb + 1) * 512)
                                    nc.tensor.matmul(
                                        sA[:, hs], kT[0:64, fc, kc * 128:(kc + 1) * 128],
                                        qT[0:64, fc, qsl],
                                        start=True, stop=True, tile_position=(0, 0),
                                    )
                                    nc.tensor.matmul(
                                        sB[:, hs], kT[64:128, fc, kc * 128:(kc + 1) * 128],
                                        qT[64:128, fc, qsl],
                                        start=True, stop=True, tile_position=(64, 0),
                                    )
                                aA = work.tile([128, 1024], BF16, tag="aA")
                                aB = work.tile([128, 1024], BF16, tag="aB")
                                jj = it - 2 * qb
                                e_off = 256 if jj == 1 else 0
                                nc.scalar.activation(
                                    aA[:, e_off:], sA[:, e_off:], AF.Exp, scale=0.125)
                                nc.scalar.activation(
                                    aB[:, e_off:], sB[:, e_off:], AF.Exp, scale=0.125)
                                if jj in (0, 1):
                                    for half in range(2):
                                        j = 2 * jj + half
                                        col = half * 512 + 128 * j
                                        cs = slice(col, col + 128)
                                        nc.vector.tensor_mul(aA[:, cs], aA[:, cs], dmask[:])
                                        nc.vector.tensor_mul(aB[:, cs], aB[:, cs], dmask[:])
                                aAs[it], aBs[it] = aA, aB
                                if qb == 0 and fc == 0 and it == 0:
                                    # cover the first-exp ramp bubble: PE has
                                    # nothing queued while exp(it0) runs
                                    pops(2)
                            if it >= 1:
                                pa, pb = aAs.pop(it - 1), aBs.pop(it - 1)
                                for half in range(2):
                                    kc = 2 * (it - 1) + half
                                    j = kc - 4 * qb
                                    off = 128 * j if j > 0 else 0
                                    hs = slice(half * 512 + off, (half + 1) * 512)
                                    nc.tensor.matmul(
                                        oA[:, off:512], vv[:, kc, 2 * fc, :], pa[:, hs],
                                        start=(kc == 0), stop=(kc == nkc - 1),
                                    )
                                    nc.tensor.matmul(
                                        oB[:, off:512], vv[:, kc, 2 * fc + 1, :], pb[:, hs],
                                        start=(kc == 0), stop=(kc == nkc - 1),
                                    )
                                pops(npop_for(qb, fc, it))
                        # normalize: one shared reciprocal for both head
                        # halves (sums stacked [128,512]) then two mults
                        sums = work.tile([128, 512], F32, tag="sums")
                        nc.vector.tensor_copy(sums[0:64, :], oA[64:128, :])
                        nc.vector.tensor_copy(sums[64:128, :], oB[64:128, :])
                        rec = work.tile([128, 512], F32, tag="rec")
                        nc.vector.reciprocal_approx_fast(rec[:], sums[:])
                        nc.vector.tensor_tensor(
                            oT[0:64, fc, qs], oA[0:64, :], rec[0:64, :], ALU.mult)
                        nc.vector.tensor_tensor(
                            oT[64:128, fc, qs], oB[0:64, :], rec[64:128, :], ALU.mult)
                    for t in range(4 * qb, 4 * qb + 4):
                        filler.append((emit_proj_half, (t, 0)))
                        filler.append((emit_proj_half, (t, 1)))
                drain_mode[0] = True
                pops(len(filler))

    nc.compile()
    return nc


def kernel(x, w_qkv, w_proj):
    global _nc_cache, LAST_EXEC_NS
    if _nc_cache is None:
        _install_ntff_hook()
        _nc_cache = _build_nc()
    nc = _nc_cache

    x = np.asarray(x)
    w_qkv = np.asarray(w_qkv)
    w_proj = np.asarray(w_proj)
    b = x.shape[0]

    # reference column layout: qkv[..., h, d, j] = w_qkv col h*192 + d*3 + j
    d_idx = np.arange(HD)
    in_maps = []
    for c in range(NCORES):
        bi, hg = divmod(c, HPC)
        heads = np.arange(HPC * hg, HPC * hg + HPC)
        qcols = (heads[:, None] * (3 * HD) + d_idx[None, :] * 3).reshape(-1)
        prows = (heads[:, None] * HD + d_idx[None, :]).reshape(-1)
        def chunked(w, c):  # [c*128, f] -> [128, c*f] (4KB-contiguous rows)
            f = w.shape[1]
            return np.ascontiguousarray(
                w.reshape(c, 128, f).transpose(1, 0, 2).reshape(128, c * f))
        in_maps.append({
            "x": np.ascontiguousarray(x[bi].T).astype(_BF16),
            "wq": chunked(w_qkv[:, qcols], KC).astype(_BF16),
            "wk": chunked(w_qkv[:, qcols + 1], KC).astype(_BF16),
            "wv": chunked(w_qkv[:, qcols + 2], KC).astype(_BF16),
            "wp": chunked(w_proj[prows, :], 2).astype(_BF16),
        })

    trace = bool(os.environ.get("BASS_TRACE"))
    res = run_bass_kernel_spmd(nc, in_maps, list(range(NCORES)), trace=trace)
    LAST_EXEC_NS = res.exec_time_ns

    out = np.zeros((b, N, D), np.float32)
    for c in range(NCORES):
        out[c // HPC] += res.results[c]["y"].astype(np.float32)
    return out


# revision 22
# speedup vs baseline: 1.0334x; 1.0334x over previous
"""Causal self-attention (b=2, n=2048, d_model=1024, 16 heads) on 8 TRN2 cores.

Sharding: core c handles batch c//4, heads 4*(c%4) .. 4*(c%4)+3 (data parallel
over batch x tensor parallel over heads). Each core computes its 4 heads'
attention and a partial output projection; the host sums the 4 partials per
batch (partials stored bf16, summed in f32 on host).

Device pipeline per core (all matmuls bf16 with fp32 PSUM accumulation):
  A. x^T arrives host-transposed; qT/kT [2x128, 2048] = W^T x^T feature-major;
     V [2048, 4x64] token-major padded with 64 ones-columns per head so the
     AV matmul also emits replicated softmax row-sums.
  B. Attention runs qb-outer (512-query blocks), fc-inner (head pairs).
     S^T tiles [128k, 512q] via row-paired K=64 matmuls; causally-trimmed:
     diagonal key chunks only compute q >= chunk start (N = 512-128j), the
     per-element diagonal triangle handled by a single [128,128] tril mask
     multiplied into the 128x128 diagonal blocks of the prob tiles.
     exp on ScalarE (scale=1/8, no max-subtraction: |scores/8| << 88).
  C. PE filler (remaining qk/V groups, then per-token-block projection units)
     keeps the PE stream dense while ACT paces the S->exp->AV chain.
  D. Projection per 128-token block t: Y_t = oT_t.T @ Wp (both 512-col
     halves), cast bf16, one DMA per block on the idle Sync queue.
"""
import contextlib
import ctypes
import os
import sys
import types

import numpy as np
import ml_dtypes

import concourse.bacc as bacc
import concourse.tile as tile
from concourse import mybir
from concourse.bass_utils import run_bass_kernel_spmd

F32 = mybir.dt.float32
BF16 = mybir.dt.bfloat16
AF = mybir.ActivationFunctionType
ALU = mybir.AluOpType

N = 2048          # sequence length
D = 1024          # d_model
NH = 16           # total heads
HD = 64           # head dim
HPC = 4           # heads per core
NCORES = 8
TC = N // 128     # token chunks (16)
KC = D // 128     # d_model chunks (8)
QB = N // 512     # 512-wide q blocks (4)

_BF16 = ml_dtypes.bfloat16

_nc_cache = None
LAST_EXEC_NS = None


def _install_ntff_hook():
    """bass_utils wants antenv.axon_hooks for trace=True under axon; the agent
    image lacks it. Synthesize it (same ctypes recipe trn_agent_boot uses)."""
    if "antenv.axon_hooks" in sys.modules:
        return
    so_path = "/opt/axon/libaxon_pjrt.so"
    try:
        lib = ctypes.CDLL(so_path)
        lib.axon_start_nrt_profile.argtypes = [
            ctypes.POINTER(ctypes.c_int64), ctypes.c_size_t]
        lib.axon_start_nrt_profile.restype = ctypes.c_int64
        lib.axon_stop_nrt_profile.argtypes = [ctypes.c_char_p]
        lib.axon_stop_nrt_profile.restype = ctypes.c_int64
    except OSError:
        return

    @contextlib.contextmanager
    def _hook(output_dir, device_ids):
        import jax
        jax.devices()
        if device_ids:
            ids = (ctypes.c_int64 * len(device_ids))(*device_ids)
            rc = lib.axon_start_nrt_profile(ids, len(device_ids))
        else:
            rc = lib.axon_start_nrt_profile(None, 0)
        if rc != 0:
            raise RuntimeError(f"axon_start_nrt_profile rc={rc}")
        try:
            yield
        finally:
            n = lib.axon_stop_nrt_profile(str(output_dir).encode())
            print(f"ntff profile: {n} file(s) -> {output_dir}", file=sys.stderr)

    mod = types.ModuleType("antenv.axon_hooks")
    mod.get_axon_ntff_profile_hook = lambda: _hook
    mod.set_axon_ntff_profile_hook = lambda h: None
    sys.modules["antenv.axon_hooks"] = mod
    try:
        import antenv
        antenv.axon_hooks = mod
    except ImportError:
        pass


def _build_nc():
    nc = bacc.Bacc("TRN2", target_bir_lowering=False, debug=False)
    # weights arrive host-pre-chunked as [128, c*f] so the DMA reads 4KB
    # contiguous per partition row (512B rows ran at ~84GB/s, 4x slower)
    x_d = nc.dram_tensor("x", [D, N], BF16, kind="ExternalInput")  # x^T, host-transposed
    wq_d = nc.dram_tensor("wq", [128, KC * HPC * HD], BF16, kind="ExternalInput")
    wk_d = nc.dram_tensor("wk", [128, KC * HPC * HD], BF16, kind="ExternalInput")
    wv_d = nc.dram_tensor("wv", [128, KC * HPC * HD], BF16, kind="ExternalInput")
    wp_d = nc.dram_tensor("wp", [128, 2 * D], BF16, kind="ExternalInput")
    y_d = nc.dram_tensor("y", [N, D], BF16, kind="ExternalOutput")

    with tile.TileContext(nc) as tc:
        with (
            tc.tile_pool(name="const", bufs=1) as constp,
            tc.tile_pool(name="big", bufs=1) as big,
            tc.tile_pool(name="work", bufs=3) as work,
        ):
            # persistent SBUF tensors
            xT = big.tile([128, KC, N], BF16, tag="xT")          # xT[p,d,t] = x[t, d*128+p]
            qT = big.tile([128, 2, N], BF16, tag="qT")           # [head-pair chunk][feat, tok]
            kT = big.tile([128, 2, N], BF16, tag="kT")
            vv = big.tile([128, TC, HPC, 128], BF16, tag="vv")   # V' per head: 64 V cols + 64 ones
            oT = big.tile([128, 2, N], BF16, tag="oT")           # normalized O^T
            wq_s = big.tile([128, KC, HPC * HD], BF16, tag="wq")
            wk_s = big.tile([128, KC, HPC * HD], BF16, tag="wk")
            wv_s = big.tile([128, KC, HPC * HD], BF16, tag="wv")
            wp_s = big.tile([128, 2, D], BF16, tag="wp")

            # Input DMAs are HBM-bandwidth-bound (~16us total); order the wire
            # stream so the chunk-major prologue can chase x chunks: qkv
            # weights first, then x chunks round-robin on the two HWDGE
            # queues, wp (needed late) last on gpsimd's SW queue.
            x_r = x_d.ap().rearrange("(c p) t -> p c t", p=128)
            nc.sync.dma_start(wq_s[:], wq_d.ap().rearrange("p (c f) -> p c f", c=KC))
            nc.scalar.dma_start(wk_s[:], wk_d.ap().rearrange("p (c f) -> p c f", c=KC))
            nc.gpsimd.dma_start(wv_s[:], wv_d.ap().rearrange("p (c f) -> p c f", c=KC))
            for c in range(KC):
                eng = nc.sync if c % 2 == 0 else nc.scalar
                eng.dma_start(xT[:, c, :], x_r[:, c, :])
            nc.gpsimd.memset(vv[:, :, :, HD:], 1.0)  # ones columns -> replicated row-sums

            # single [128,128] tril mask for the per-element diagonal blocks:
            # dmask[k, q] = 1 iff q >= k
            dmask = constp.tile([128, 128], BF16, tag="dmask")
            nc.gpsimd.memset(dmask[:], 1.0)
            nc.gpsimd.affine_select(
                out=dmask[:], in_=dmask[:], compare_op=ALU.is_ge, fill=0.0,
                base=0, pattern=[[1, 128]], channel_multiplier=-1,
            )
            nc.gpsimd.dma_start(wp_s[:], wp_d.ap().rearrange("p (c f) -> p c f", c=2))

            # ---- chunk-major prologue: everything attention(qb0) needs ----
            # 8 accumulation groups live in 8 PSUM banks simultaneously; the
            # per-chunk MMs chase the x DMA arrivals instead of head-blocking
            # the in-order PE queue on the last chunk.
            # two pools so closing them staggers the WAR deps: the attention
            # psS pool only waits on the qk copies, not the v copies
            with (
                tc.tile_pool(name="psPq", bufs=1, space="PSUM") as psPq,
                tc.tile_pool(name="psPv", bufs=1, space="PSUM") as psPv,
            ):
                pq = [psPq.tile([128, 512], F32, tag=f"pq{i}", name=f"pq{i}")
                      for i in range(4)]
                pv = [psPv.tile([128, HPC * HD], F32, tag=f"pv{i}", name=f"pv{i}")
                      for i in range(3)]
                qk_list = [(wq_s, 0), (wk_s, 0), (wq_s, 1), (wk_s, 1)]
                for kc in range(KC):
                    for i, (ws, fc) in enumerate(qk_list):
                        nc.tensor.matmul(
                            pq[i][:], ws[:, kc, fc * 128:(fc + 1) * 128],
                            xT[:, kc, 0:512],
                            start=(kc == 0), stop=(kc == KC - 1),
                        )
                    for t in range(3):
                        nc.tensor.matmul(
                            pv[t][:], xT[:, kc, t * 128:(t + 1) * 128],
                            wv_s[:, kc, :],
                            start=(kc == 0), stop=(kc == KC - 1),
                        )
                # fc0 q/k copies first (gate attention start) on DVE; fc1
                # copies on the idle ScalarE; v copies on DVE behind fc0's
                nc.vector.tensor_copy(qT[:, 0, 0:512], pq[0][:])
                nc.vector.tensor_copy(kT[:, 0, 0:512], pq[1][:])
                nc.scalar.activation(qT[:, 1, 0:512], pq[2][:], AF.Copy)
                nc.scalar.activation(kT[:, 1, 0:512], pq[3][:], AF.Copy)
                for t in range(3):
                    nc.vector.tensor_copy(
                        vv[:, t, :, 0:HD],
                        pv[t][:].rearrange("p (h d) -> p h d", h=HPC),
                    )

            with (
                tc.tile_pool(name="psS", bufs=2, space="PSUM") as psS,
                tc.tile_pool(name="psO", bufs=2, space="PSUM") as psO,
                tc.tile_pool(name="psF", bufs=2, space="PSUM") as psF,
            ):
                def emit_qk_group(fc, tb, wsrc, dst):
                    ps = psF.tile([128, 512], F32, tag="fy")
                    for kc in range(KC):
                        nc.tensor.matmul(
                            ps[:], wsrc[:, kc, fc * 128:(fc + 1) * 128],
                            xT[:, kc, tb * 512:(tb + 1) * 512],
                            start=(kc == 0), stop=(kc == KC - 1),
                        )
                    nc.vector.tensor_copy(dst[:, fc, tb * 512:(tb + 1) * 512], ps[:])

                def emit_v_group(t):
                    ps = psF.tile([128, HPC * HD], F32, tag="fy")
                    for kc in range(KC):
                        nc.tensor.matmul(
                            ps[:], xT[:, kc, t * 128:(t + 1) * 128], wv_s[:, kc, :],
                            start=(kc == 0), stop=(kc == KC - 1),
                        )
                    nc.vector.tensor_copy(
                        vv[:, t, :, 0:HD],
                        ps[:].rearrange("p (h d) -> p h d", h=HPC),
                    )

                drain_mode = [False]
                ys_live = {}

                def emit_proj_half(t, nh):
                    # one 512-col half of Y for token block t; DMA after nh=1
                    if nh == 0:
                        ys_live[t] = work.tile([128, D], BF16, tag="ys", name="ys")
                    ys = ys_live[t]
                    ps = psF.tile([128, 512], F32, tag="fy")
                    for fc in range(2):
                        nc.tensor.matmul(
                            ps[:], oT[:, fc, t * 128:(t + 1) * 128],
                            wp_s[:, fc, nh * 512:(nh + 1) * 512],
                            start=(fc == 0), stop=(fc == 1),
                        )
                    dst = ys[:, nh * 512:(nh + 1) * 512]
                    if drain_mode[0]:
                        # post-attention drain: ScalarE is idle
                        nc.scalar.activation(dst, ps[:], AF.Copy)
                    else:
                        nc.vector.tensor_copy(dst, ps[:])
                    if nh == 1:
                        del ys_live[t]
                        nc.sync.dma_start(
                            y_d.ap()[t * 128:(t + 1) * 128, :], ys[:])

                # filler ordered so every entry is emitted before its first
                # consumer under the per-(qb,fc) pop counts in NPOP below
                filler = [
                    (emit_v_group, (3,)),
                    (emit_qk_group, (0, 1, wq_s, qT)),
                    (emit_qk_group, (0, 1, wk_s, kT)),
                    (emit_qk_group, (1, 1, wq_s, qT)),
                    (emit_qk_group, (1, 1, wk_s, kT)),
                    (emit_v_group, (4,)), (emit_v_group, (5,)),
                    (emit_v_group, (6,)), (emit_v_group, (7,)),
                    (emit_qk_group, (0, 2, wq_s, qT)),
                    (emit_qk_group, (0, 2, wk_s, kT)),
                    (emit_qk_group, (1, 2, wq_s, qT)),
                    (emit_qk_group, (1, 2, wk_s, kT)),
                    (emit_v_group, (8,)), (emit_v_group, (9,)),
                    (emit_v_group, (10,)), (emit_v_group, (11,)),
                    (emit_qk_group, (0, 3, wq_s, qT)),
                    (emit_qk_group, (0, 3, wk_s, kT)),
                    (emit_v_group, (12,)), (emit_v_group, (13,)),
                    (emit_v_group, (14,)), (emit_v_group, (15,)),
                    (emit_qk_group, (1, 3, wq_s, qT)),
                    (emit_qk_group, (1, 3, wk_s, kT)),
                ]

                def pops(n):
                    for _ in range(min(n, len(filler))):
                        fn, args = filler.pop(0)
                        fn(*args)

                # pops per AV-iteration, keyed (qb, fc).  Tuned so the filler
                # stream covers the ACT-paced deficit early, does not run dry
                # during qb3, and leaves ~3 proj halves for the drain: those
                # are emitted after the final normalize but do not depend on
                # it, so PE stays busy (and the HAM clock warm) through it.
                def npop_for(qb, fc, it):
                    if (qb, fc) == (3, 1):
                        # leave ~5 proj halves in stock: the drain emits them
                        # after the final normalize, and they don't depend on
                        # it, so PE stays busy through its ~3.5us DVE chain
                        return 2 if 4 <= it <= 7 else 0
                    if qb == 0:
                        return 2
                    return 1

                for qb in range(QB):
                    nkc = 4 * (qb + 1)
                    npr = nkc // 2
                    qs = slice(qb * 512, (qb + 1) * 512)
                    for fc in range(2):
                        oA = psO.tile([128, 512], F32, tag="oy")
                        oB = psO.tile([128, 512], F32, tag="oy")
                        aAs, aBs = {}, {}
                        for it in range(npr + 1):
                            if it < npr:
                                sA = psS.tile([128, 1024], F32, tag="s")
                                sB = psS.tile([128, 1024], F32, tag="s")
                                for half in range(2):
                                    kc = 2 * it + half
                                    j = kc - 4 * qb
                                    off = 128 * j if j > 0 else 0
                                    hs = slice(half * 512 + off, (half + 1) * 512)
                                    qsl = slice(qb * 512 + off, (qb + 1) * 512)
                                    nc.tensor.matmul(
                                        sA[:, hs], kT[0:64, fc, kc * 128:(kc + 1) * 128],
                                        qT[0:64, fc, qsl],
                                        start=True, stop=True, tile_position=(0, 0),
                                    )
                                    nc.tensor.matmul(
                                        sB[:, hs], kT[64:128, fc, kc * 128:(kc + 1) * 128],
                                        qT[64:128, fc, qsl],
                                        start=True, stop=True, tile_position=(64, 0),
                                    )
                                aA = work.tile([128, 1024], BF16, tag="aA")
                                aB = work.tile([128, 1024], BF16, tag="aB")
                                jj = it - 2 * qb
                                e_off = 256 if jj == 1 else 0
                                nc.scalar.activation(
                                    aA[:, e_off:], sA[:, e_off:], AF.Exp, scale=0.125)
                                nc.scalar.activation(
                                    aB[:, e_off:], sB[:, e_off:], AF.Exp, scale=0.125)
                                if jj in (0, 1):
                                    for half in range(2):
                                        j = 2 * jj + half
                                        col = half * 512 + 128 * j
                                        cs = slice(col, col + 128)
                                        nc.vector.tensor_mul(aA[:, cs], aA[:, cs], dmask[:])
                                        nc.vector.tensor_mul(aB[:, cs], aB[:, cs], dmask[:])
                                aAs[it], aBs[it] = aA, aB
                                if qb == 0 and fc == 0 and it == 0:
                                    # cover the first-exp ramp bubble: PE has
                                    # nothing queued while exp(it0) runs
                                    pops(2)
                            if it >= 1:
                                pa, pb = aAs.pop(it - 1), aBs.pop(it - 1)
                                for half in range(2):
                                    kc = 2 * (it - 1) + half
                                    j = kc - 4 * qb
                                    off = 128 * j if j > 0 else 0
                                    hs = slice(half * 512 + off, (half + 1) * 512)
                                    nc.tensor.matmul(
                                        oA[:, off:512], vv[:, kc, 2 * fc, :], pa[:, hs],
                                        start=(kc == 0), stop=(kc == nkc - 1),
                                    )
                                    nc.tensor.matmul(
                                        oB[:, off:512], vv[:, kc, 2 * fc + 1, :], pb[:, hs],
                                        start=(kc == 0), stop=(kc == nkc - 1),
                                    )
                                pops(npop_for(qb, fc, it))
                        # normalize: one shared reciprocal for both head
                        # halves (sums stacked [128,512]) then two mults
                        sums = work.tile([128, 512], F32, tag="sums")
                        nc.vector.tensor_copy(sums[0:64, :], oA[64:128, :])
                        nc.vector.tensor_copy(sums[64:128, :], oB[64:128, :])
                        rec = work.tile([128, 512], F32, tag="rec")
                        nc.vector.reciprocal_approx_fast(rec[:], sums[:])
                        nc.vector.tensor_tensor(
                            oT[0:64, fc, qs], oA[0:64, :], rec[0:64, :], ALU.mult)
                        nc.vector.tensor_tensor(
                            oT[64:128, fc, qs], oB[0:64, :], rec[64:128, :], ALU.mult)
                    for t in range(4 * qb, 4 * qb + 4):
                        filler.append((emit_proj_half, (t, 0)))
                        filler.append((emit_proj_half, (t, 1)))
                drain_mode[0] = True
                pops(len(filler))

    nc.compile()
    return nc


def kernel(x, w_qkv, w_proj):
    global _nc_cache, LAST_EXEC_NS
    if _nc_cache is None:
        _install_ntff_hook()
        _nc_cache = _build_nc()
    nc = _nc_cache

    x = np.asarray(x)
    w_qkv = np.asarray(w_qkv)
    w_proj = np.asarray(w_proj)
    b = x.shape[0]

    # reference column layout: qkv[..., h, d, j] = w_qkv col h*192 + d*3 + j
    d_idx = np.arange(HD)
    in_maps = []
    for c in range(NCORES):
        bi, hg = divmod(c, HPC)
        heads = np.arange(HPC * hg, HPC * hg + HPC)
        qcols = (heads[:, None] * (3 * HD) + d_idx[None, :] * 3).reshape(-1)
        prows = (heads[:, None] * HD + d_idx[None, :]).reshape(-1)
        def chunked(w, c):  # [c*128, f] -> [128, c*f] (4KB-contiguous rows)
            f = w.shape[1]
            return np.ascontiguousarray(
                w.reshape(c, 128, f).transpose(1, 0, 2).reshape(128, c * f))
        in_maps.append({
            "x": np.ascontiguousarray(x[bi].T).astype(_BF16),
            "wq": chunked(w_qkv[:, qcols], KC).astype(_BF16),
            "wk": chunked(w_qkv[:, qcols + 1], KC).astype(_BF16),
            "wv": chunked(w_qkv[:, qcols + 2], KC).astype(_BF16),
            "wp": chunked(w_proj[prows, :], 2).astype(_BF16),
        })

    trace = bool(os.environ.get("BASS_TRACE"))
    res = run_bass_kernel_spmd(nc, in_maps, list(range(NCORES)), trace=trace)
    LAST_EXEC_NS = res.exec_time_ns

    out = np.zeros((b, N, D), np.float32)
    for c in range(NCORES):
        out[c // HPC] += res.results[c]["y"].astype(np.float32)
    return out
